# revision 1
# baseline (speedup 1.0000x reference)
"""Trainium2 Bass kernel for nn_EnetGnn (GNN message passing).

Reference computation (per batch n, with X = rgb_in[n] viewed as (C=1024, HW=1024),
nodes = columns of X):
  S[i,j]   = x_i . x_j                       (node similarity)
  nb(i)    = 16 smallest entries of S[i,:]   (k-NN, torch topk largest=False)
  M[m,:]   = relu(relu(X0_node_m @ w1 + b1) @ w2 + b2)   (MLP table; the
             reference gathers from the *globally flattened* node table, i.e.
             always batch 0's nodes)
  g_i      = mean_{m in nb(i)} M[m,:]
  A[i,j]   = g_i . g_j ; softmax over axis i (columns normalized)
  out      = X @ A_softmax + X

Implementation (8 cores, SPMD, one compiled program): core c handles batch
n = c//2 and channel-half h = c%2.  Each core computes the full
S/topk/MLP/G/A pipeline for its batch (duplicated within the pair) and the
final output for its 512-channel half.  (A pair-AllGather split of the
front half was tried; the 2x1MB collective costs 30-50us in this runtime,
more than the duplicated compute.)
  - fp32 matmul is 4x the cost of bf16 on trn2 (LOW_HIGH two-pass), so all
    matmuls run in bf16 with f32 psum accumulation.  Validated numerically:
    the top-16 sets see ~80/65536 boundary flips, each worth ~1e-3 absolute
    on the output (rel err 4e-4 overall).
  - top-16 per row via DVE max8 + match_replace (2 rounds), mask via not_equal
  - neighbor mean as a matmul with the 0/1 mask (P^T), M scaled by 1/16
  - softmax over the partition axis: exp on ACT, column sums via ones-vector
    matmul on PE, 1/colsum = exp(-ln(cs)) on ACT, partition-broadcast on
    GpSimd, normalization applied after the output matmul.
"""

import numpy as np
from contextlib import ExitStack

from concourse import mybir, bacc, tile
from concourse.bass import ts
from concourse.bass_utils import run_bass_kernel_spmd
from concourse.masks import make_identity

F32 = mybir.dt.float32
BF16 = mybir.dt.bfloat16
P = 128
HWDIM = 1024   # number of nodes per batch (H*W)
CDIM = 1024    # channels
FDIM = 256     # MLP hidden dim
NB = 4         # batch
NCORES = 8
JH = HWDIM // 2  # nodes owned per core (columns rolled to front)
MINVAL = -1.0e30

Copy = mybir.ActivationFunctionType.Copy
Relu = mybir.ActivationFunctionType.Relu
Exp = mybir.ActivationFunctionType.Exp
Ln = mybir.ActivationFunctionType.Ln


def _build_program(nc: bacc.Bacc, use_b2: bool):
    x = nc.dram_tensor("x", [CDIM, HWDIM], F32, kind="ExternalInput").ap()
    xj = nc.dram_tensor("xj", [CDIM, JH], F32, kind="ExternalInput").ap()
    x0 = nc.dram_tensor("x0", [CDIM, HWDIM], F32, kind="ExternalInput").ap()
    w1 = nc.dram_tensor("w1", [CDIM, FDIM], F32, kind="ExternalInput").ap()
    w2 = nc.dram_tensor("w2", [FDIM, CDIM], F32, kind="ExternalInput").ap()
    b1 = nc.dram_tensor("b1", [2, P, 1], F32, kind="ExternalInput").ap()
    b2 = nc.dram_tensor("b2", [1, CDIM], F32, kind="ExternalInput").ap()
    out = nc.dram_tensor("out", [CDIM, JH], F32, kind="ExternalOutput").ap()

    with tile.TileContext(nc) as tc, ExitStack() as ctx:
        persist = ctx.enter_context(tc.tile_pool(name="persist", bufs=1))

        # ---- constants ----
        id_b = persist.tile([P, P], BF16, tag="id_b", name="id_b")
        make_identity(nc, id_b[:])
        ones_row = persist.tile([1, P], F32, tag="ones_row", name="ones_row")
        nc.vector.memset(ones_row[:], 1.0)
        ones_col_b = persist.tile([P, 1], BF16, tag="ones_col_b", name="ones_col_b")
        nc.vector.memset(ones_col_b[:], 1.0)

        # ---- persistent sbuf buffers ----
        xj_sb = [persist.tile([P, JH], F32, tag=f"xj{i}", name=f"xj{i}")
                 for i in range(8)]
        w1b = [persist.tile([P, FDIM], BF16, tag=f"w1b{i}", name=f"w1b{i}")
               for i in range(8)]
        w2b = [persist.tile([P, CDIM], BF16, tag=f"w2b{i}", name=f"w2b{i}")
               for i in range(2)]
        b1t = [persist.tile([P, 1], F32, tag=f"b1t{i}", name=f"b1t{i}")
               for i in range(2)]
        b2row = persist.tile([1, CDIM], F32, tag="b2row", name="b2row")
        h1t = [persist.tile([P, HWDIM], BF16, tag=f"h1t{i}", name=f"h1t{i}")
               for i in range(2)]
        m_sb = [persist.tile([P, CDIM], BF16, tag=f"m{i}", name=f"m{i}")
                for i in range(8)]
        pt_sb = [persist.tile([P, HWDIM], BF16, tag=f"pt{i}", name=f"pt{i}")
                 for i in range(8)]
        r_sb = [persist.tile([P, CDIM], BF16, tag=f"r{i}", name=f"r{i}")
                for i in range(8)]
        gt_sb = [persist.tile([P, HWDIM], BF16, tag=f"gt{i}", name=f"gt{i}")
                 for i in range(8)]

        with ExitStack() as s1:
            sx = s1.enter_context(tc.tile_pool(name="sx", bufs=1))
            xb = [sx.tile([P, HWDIM], BF16, tag=f"xb{i}", name=f"xb{i}")
                  for i in range(8)]
            x0b = [sx.tile([P, HWDIM], BF16, tag=f"x0b{i}", name=f"x0b{i}")
                   for i in range(8)]
            pmask = [sx.tile([P, HWDIM], BF16, tag=f"pm{i}", name=f"pm{i}")
                     for i in range(8)]
            stream = s1.enter_context(tc.tile_pool(name="stream", bufs=3))
            topk_pool = s1.enter_context(tc.tile_pool(name="topk", bufs=3))

            # x tiles first (S starts as soon as tile 0 lands + casts);
            # alternate cast engine so the cast chain isn't ACT-serial
            for i in range(8):
                xf = stream.tile([P, HWDIM], F32, tag="xf", name="xf")
                nc.sync.dma_start(xf[:], x[ts(i, P), :])
                if i % 2 == 0:
                    nc.scalar.activation(xb[i][:], xf[:], Copy)
                else:
                    nc.vector.tensor_copy(out=xb[i][:], in_=xf[:])
            # x0 prefetched + cast during the S stage
            for i in range(8):
                x0f = stream.tile([P, HWDIM], F32, tag="xf", name="x0f")
                nc.sync.dma_start(x0f[:], x0[ts(i, P), :])
                if i % 2 == 0:
                    nc.scalar.activation(x0b[i][:], x0f[:], Copy)
                else:
                    nc.vector.tensor_copy(out=x0b[i][:], in_=x0f[:])
            for i in range(8):
                nc.sync.dma_start(xj_sb[i][:], xj[ts(i, P), :])
            for i in range(8):
                wf = stream.tile([P, FDIM], F32, tag="wf", name="wf", bufs=2)
                nc.sync.dma_start(wf[:], w1[ts(i, P), :])
                nc.scalar.activation(w1b[i][:], wf[:], Copy)
            for i in range(2):
                w2f = stream.tile([P, CDIM], F32, tag="w2f", name="w2f", bufs=2)
                nc.sync.dma_start(w2f[:], w2[ts(i, P), :])
                nc.scalar.activation(w2b[i][:], w2f[:], Copy)
            for i in range(2):
                nc.sync.dma_start(b1t[i][:], b1[i])
            nc.sync.dma_start(b2row[:], b2[:, :])

            with ExitStack() as ps1:
                # ps_s opens FIRST so that ps_t/ps_g later reuse the
                # MLP/R pools' banks (free at ~55us) instead of the S
                # banks, whose release-dep is the LAST topk read (~69us)
                ps_s = ps1.enter_context(
                    tc.tile_pool(name="ps_s", bufs=2, space="PSUM"))
                ps_mr_scope = ExitStack()
                ps_hm = ps_mr_scope.enter_context(
                    tc.tile_pool(name="ps_hm", bufs=2, space="PSUM"))
                ps_r = ps_mr_scope.enter_context(
                    tc.tile_pool(name="ps_r", bufs=2, space="PSUM"))

                # ---- stages 1+2 interleaved: S tiles are paced by the DVE
                # topk chain, so MLP matmul chunks are emitted between S
                # tiles to keep the in-order PE stream busy and have the M
                # table ready the moment the last P^T transpose lands ----
                def mlp_h_chunk(k):
                    ft, ih = k // 2, k % 2
                    ps = ps_hm.tile([P, 512], F32, tag="HM", name="hps")
                    for cc in range(8):
                        nc.tensor.matmul(
                            ps[:], w1b[cc][:, ts(ft, P)], x0b[cc][:, ts(ih, 512)],
                            start=(cc == 0), stop=(cc == 7),
                        )
                    nc.scalar.activation(
                        h1t[ft][:, ts(ih, 512)], ps[:], Relu, bias=b1t[ft][:],
                    )

                def mlp_m_chunk(k):
                    mt, chh = k // 2, k % 2
                    ps = ps_hm.tile([P, 512], F32, tag="HM", name="mps")
                    nc.tensor.matmul(ps[:], h1t[0][:, ts(mt, P)],
                                     w2b[0][:, ts(chh, 512)],
                                     start=True, stop=not use_b2)
                    nc.tensor.matmul(ps[:], h1t[1][:, ts(mt, P)],
                                     w2b[1][:, ts(chh, 512)],
                                     start=False, stop=False,
                                     skip_group_check=True)
                    if use_b2:
                        # + b2 broadcast along partitions via rank-1 matmul
                        nc.tensor.matmul(ps[:], ones_row[:],
                                         b2row[0:1, ts(chh, 512)],
                                         start=False, stop=True)
                    # relu(ps)/16 == relu(ps/16)
                    nc.scalar.activation(
                        m_sb[mt][:, ts(chh, 512)], ps[:], Relu, scale=1.0 / 16.0,
                    )

                def r_chunk(mt):
                    # R (m, c) = transpose of bf16 x; consumed late by OUT
                    psr = ps_r.tile([P, CDIM], BF16, tag="R", name="psr")
                    for cq in range(8):
                        nc.tensor.transpose(
                            psr[:, ts(cq, P)], xb[cq][:, ts(mt, P)], id_b[:])
                    nc.scalar.activation(r_sb[mt][:], psr[:], Copy)

                # chunk order respects data readiness: H chunks ih-major so
                # M chunks for mt<4 (needing the ih=0 halves) can follow
                # early.  R chunks are NOT interleaved: their ACT psum
                # copies would queue ahead of the sneg negates that feed
                # the DVE topk pacer (DVE is only ~70% busy in its window)
                H, M = mlp_h_chunk, mlp_m_chunk
                mlp_chunks = (
                    [lambda: H(0), lambda: H(2), lambda: H(1), lambda: H(3)]
                    + [lambda k=k: M(k) for k in range(16)]
                )
                # chunks emitted after S tile t (none before tile 1: x0 is
                # still streaming in)
                sched = [0, 2, 3, 3, 3, 3, 3, 3]

                for t in range(8):
                    ps = ps_s.tile([P, HWDIM], F32, tag="S")
                    for cc in range(8):
                        lhsT = xb[cc][:, ts(t, P)]
                        for jh in range(2):
                            nc.tensor.matmul(
                                ps[:, ts(jh, 512)], lhsT, xb[cc][:, ts(jh, 512)],
                                start=(cc == 0), stop=(cc == 7),
                            )
                    sneg = topk_pool.tile([P, HWDIM], F32, tag="sneg", name="sneg",
                                           bufs=4)
                    nc.scalar.activation(sneg[:], ps[:], Copy, scale=-1.0)
                    m8a = topk_pool.tile([P, 8], F32, tag="m8a", name="m8a")
                    m8b = topk_pool.tile([P, 8], F32, tag="m8b", name="m8b")
                    szap = topk_pool.tile([P, HWDIM], F32, tag="szap", name="szap")
                    nc.vector.max(out=m8a[:], in_=sneg[:])
                    nc.vector.match_replace(
                        out=szap[:], in_to_replace=m8a[:], in_values=sneg[:],
                        imm_value=MINVAL,
                    )
                    nc.vector.max(out=m8b[:], in_=szap[:])
                    nc.vector.match_replace(
                        out=szap[:], in_to_replace=m8b[:], in_values=szap[:],
                        imm_value=MINVAL,
                    )
                    # 1.0 exactly at the 16 replaced positions
                    nc.vector.tensor_tensor(
                        out=pmask[t][:], in0=sneg[:], in1=szap[:],
                        op=mybir.AluOpType.not_equal,
                    )
                    for _ in range(sched[t]):
                        mlp_chunks.pop(0)()
                for chunk in mlp_chunks:
                    chunk()
                # R transposes post-loop: PE is waiting on pmask/pool deps
                # here anyway, and their ACT copies land after the topk
                # window has mostly drained
                for mt in range(8):
                    r_chunk(mt)
                # free the S/MLP/R psum banks so the P^T transposes and G^T
                # matmuls can start right after their data deps, not after
                # the whole phase-1 flush
                ps_mr_scope.close()

                # ---- stages 3+4 interleaved by i-half: P^T-h0 -> G^T-h0
                # runs during the topk tail of tiles 4..7, then h1 ----
                ps_t = ps1.enter_context(
                    tc.tile_pool(name="ps_t", bufs=2, space="PSUM"))
                ps_g = ps1.enter_context(
                    tc.tile_pool(name="ps_g", bufs=2, space="PSUM"))
                for ih in range(2):
                    for mt in range(8):
                        ps = ps_t.tile([P, 512], BF16, tag="PT")
                        for q in range(4):
                            nc.tensor.transpose(
                                ps[:, ts(q, P)],
                                pmask[ih * 4 + q][:, ts(mt, P)], id_b[:],
                            )
                        nc.scalar.activation(pt_sb[mt][:, ts(ih, 512)], ps[:], Copy)
                    for ct in range(8):
                        ps = ps_g.tile([P, 512], F32, tag="G")
                        for mt in range(8):
                            nc.tensor.matmul(
                                ps[:], m_sb[mt][:, ts(ct, P)],
                                pt_sb[mt][:, ts(ih, 512)],
                                start=(mt == 0), stop=(mt == 7),
                            )
                        nc.vector.tensor_copy(out=gt_sb[ct][:, ts(ih, 512)],
                                              in_=ps[:])

        # ---- buffers that live only in the later stages ----
        late = ctx.enter_context(tc.tile_pool(name="late", bufs=1))
        e_sb = [late.tile([P, JH], BF16, tag=f"e{i}", name=f"e{i}")
                for i in range(8)]
        invbc = late.tile([P, JH], F32, tag="invbc", name="invbc")
        inv_row = late.tile([1, JH], F32, tag="inv_row", name="inv_row")


        # ---- stage 6: A (m, j-own) = G^T.T G^T[:, 0:512], E, column sums ----
        # inputs are in local node order (own 512 columns first), so the own
        # j-half is the static slice 0:512; mt 0..3 need only G^T half 0
        with ExitStack() as s4:
            ps_a = s4.enter_context(tc.tile_pool(name="ps_a", bufs=4, space="PSUM"))
            ps_cs = s4.enter_context(tc.tile_pool(name="ps_cs", bufs=1, space="PSUM"))
            cs = ps_cs.tile([1, JH], F32, tag="CS")
            for mt in range(8):
                ps = ps_a.tile([P, JH], F32, tag="A")
                for cc in range(8):
                    nc.tensor.matmul(
                        ps[:], gt_sb[cc][:, ts(mt, P)], gt_sb[cc][:, 0:JH],
                        start=(cc == 0), stop=(cc == 7),
                    )
                nc.scalar.activation(e_sb[mt][:], ps[:], Exp)
                nc.tensor.matmul(
                    cs[0:1, :], ones_col_b[:], e_sb[mt][:],
                    start=(mt == 0), stop=(mt == 7),
                )
            # 1/colsum = exp(-ln(colsum)) on ACT; DVE reciprocal on a
            # single partition costs ~6.5us of critical tail
            nc.scalar.activation(inv_row[0:1, :], cs[0:1, :], Ln)
            nc.scalar.activation(inv_row[0:1, :], inv_row[0:1, :], Exp, scale=-1.0)
            nc.gpsimd.partition_broadcast(invbc[:], inv_row[0:1, :], channels=P)

        # ---- stage 7: OUT = Id @ E, scale by 1/colsum, add identity ----
        with ExitStack() as s5:
            ps_o = s5.enter_context(tc.tile_pool(name="ps_o", bufs=4, space="PSUM"))
            fin_pool = s5.enter_context(tc.tile_pool(name="fin", bufs=4))
            for ct in range(8):
                ps = ps_o.tile([P, JH], F32, tag="O")
                for mt in range(8):
                    nc.tensor.matmul(
                        ps[:], r_sb[mt][:, ts(ct, P)], e_sb[mt][:],
                        start=(mt == 0), stop=(mt == 7),
                    )
                tmp = fin_pool.tile([P, JH], F32, tag="tmp", name="tmp")
                nc.vector.tensor_tensor(
                    out=tmp[:], in0=ps[:], in1=invbc[:],
                    op=mybir.AluOpType.mult)
                outt = fin_pool.tile([P, JH], F32, tag="outt", name="outt")
                nc.vector.tensor_tensor(
                    out=outt[:], in0=tmp[:], in1=xj_sb[ct][:],
                    op=mybir.AluOpType.add)
                nc.sync.dma_start(out[ts(ct, P), :], outt[:])

    return nc


_NC = {}


def _get_nc(use_b2=False):
    if use_b2 not in _NC:
        nc = bacc.Bacc("TRN2", target_bir_lowering=False, debug=False,
                       num_devices=NCORES)
        _build_program(nc, use_b2)
        nc.compile()
        _NC[use_b2] = nc
    return _NC[use_b2]


def _in_maps(cat, rgb_in, w1, b1, w2, b2):
    del cat  # unused by the reference computation
    x4 = np.ascontiguousarray(rgb_in.reshape(NB, CDIM, HWDIM)).astype(np.float32)
    w1 = np.ascontiguousarray(w1, dtype=np.float32)
    w2 = np.ascontiguousarray(w2, dtype=np.float32)
    b1r = np.ascontiguousarray(b1.reshape(2, P, 1), dtype=np.float32)
    b2r = np.ascontiguousarray(b2.reshape(1, CDIM), dtype=np.float32)
    maps = []
    for core in range(NCORES):
        n, q = core // 2, core % 2
        # local node order: this core's 512 columns first (identity for q=0)
        roll = (lambda a: a) if q == 0 else (
            lambda a: np.ascontiguousarray(np.concatenate(
                [a[:, JH:], a[:, :JH]], axis=1)))
        maps.append({
            "x": roll(x4[n]),
            "xj": np.ascontiguousarray(x4[n, :, q * JH:(q + 1) * JH]),
            "x0": roll(x4[0]),
            "w1": w1,
            "w2": w2,
            "b1": b1r,
            "b2": b2r,
        })
    return maps


def _assemble(results, rgb_shape):
    N, C, H, W = rgb_shape
    out = np.empty((N, C, H * W), np.float32)
    for core, res in enumerate(results):
        n, q = core // 2, core % 2
        out[n, :, q * JH:(q + 1) * JH] = res["out"]
    return out.reshape(N, C, H, W)


def run_on_hw(cat, rgb_in, w1, b1, w2, b2, trace=False, **kw):
    nc = _get_nc(use_b2=bool(np.any(np.asarray(b2))))
    maps = _in_maps(cat, rgb_in, w1, b1, w2, b2)
    res = run_bass_kernel_spmd(nc, maps, core_ids=list(range(NCORES)),
                               trace=trace, **kw)
    out = _assemble(res.results, rgb_in.shape)
    return out, res


def kernel(cat, rgb_in, w1, b1, w2, b2, gnn_iterations=1, k=16):
    assert int(gnn_iterations) == 1 and int(k) == 16
    cat = np.asarray(cat)
    rgb_in = np.asarray(rgb_in, dtype=np.float32)
    out, _ = run_on_hw(cat, rgb_in, np.asarray(w1), np.asarray(b1),
                       np.asarray(w2), np.asarray(b2))
    return out



# revision 2
# speedup vs baseline: 1.0238x; 1.0238x over previous
"""Trainium2 Bass kernel for nn_EnetGnn (GNN message passing) — v2 (fp8).

Reference computation (per batch n, X = rgb_in[n] as (C=1024, HW=1024),
nodes = columns of X):
  S[i,j] = x_i . x_j ; nb(i) = 16 smallest of S[i,:]
  M = relu(relu(X0_nodes @ w1 + b1) @ w2 + b2)      (batch-0 node table)
  g_i = mean_{m in nb(i)} M[m,:]
  A[i,j] = g_i . g_j ; softmax over axis i ; out = X @ A_sm + X

v2 strategy (vs the 148us bf16 baseline):
  - All big matmuls (S, MLP, G, A) run fp8e4m3 with DoubleRow perf mode
    (two 128-deep k-planes per pass).  Host uploads fp8 data already in
    [128, ksub, free] layout; no on-device input casts.
  - topk: per S-tile, DVE does max8 -> match_replace8 -> max8 on f32 -S,
    yielding the 16th-largest value; the 0/1 mask comes from ACT
    sigmoid(BIG*(sneg - thr + DELTA)) instead of two more full-width DVE
    passes (max8/match_replace have no 2x mode, so DVE paced the kernel).
    Gap stats: only ~25/4096 rows land in the sigmoid transition window.
  - Identity add folded into the output matmul: En = E*inv + I, so
    out = R^T @ En directly (kills the f32 add tail and the xj upload).
  - X^T (lhsT of the output matmul) uploaded pre-transposed in bf16.
  - PE warmup transposes during the input DMA (HAM clock gate).
  CPU sim of the full pipeline: rel err 4.1e-3 (tolerance 2e-2).
"""

import numpy as np
import ml_dtypes
from contextlib import ExitStack

from concourse import mybir, bacc, tile
from concourse.bass import ts
from concourse.bass_utils import run_bass_kernel_spmd
from concourse.masks import make_identity

F32 = mybir.dt.float32
BF16 = mybir.dt.bfloat16
FP8 = mybir.dt.float8e4
P = 128
HWDIM = 1024
CDIM = 1024
FDIM = 256
NB = 4
NCORES = 8
JH = HWDIM // 2
MINVAL = -1.0e30
BIG = 2000.0
DELTA = 0.012
DR = mybir.MatmulPerfMode.DoubleRow

Copy = mybir.ActivationFunctionType.Copy
Relu = mybir.ActivationFunctionType.Relu
Exp = mybir.ActivationFunctionType.Exp
Ln = mybir.ActivationFunctionType.Ln
Sigmoid = mybir.ActivationFunctionType.Sigmoid


def _build_program(nc: bacc.Bacc, use_b2: bool):
    # fp8 inputs in [128, ksub, free] layout (ksub = 128-row block of the
    # contraction dim)
    xq = nc.dram_tensor("xq", [P, 8, HWDIM], FP8, kind="ExternalInput").ap()
    xnq = nc.dram_tensor("xnq", [P, 8, HWDIM], FP8, kind="ExternalInput").ap()
    x0q = nc.dram_tensor("x0q", [P, 8, HWDIM], FP8, kind="ExternalInput").ap()
    w1q = nc.dram_tensor("w1q", [P, 8, FDIM], FP8, kind="ExternalInput").ap()
    w2q = nc.dram_tensor("w2q", [P, 2, CDIM], FP8, kind="ExternalInput").ap()
    rt = nc.dram_tensor("rt", [HWDIM, CDIM], BF16, kind="ExternalInput").ap()
    b1 = nc.dram_tensor("b1", [2, P, 1], F32, kind="ExternalInput").ap()
    b2 = nc.dram_tensor("b2", [1, CDIM], BF16, kind="ExternalInput").ap()
    out = nc.dram_tensor("out", [CDIM, JH], F32, kind="ExternalOutput").ap()

    with tile.TileContext(nc) as tc, ExitStack() as ctx:
        persist = ctx.enter_context(tc.tile_pool(name="persist", bufs=1))

        id_b = persist.tile([P, P], BF16, tag="id_b", name="id_b")
        make_identity(nc, id_b[:])
        ones_col_b = persist.tile([P, 1], BF16, tag="ones_col_b",
                                  name="ones_col_b")
        nc.vector.memset(ones_col_b[:], 1.0)
        if use_b2:
            ones_row_b = persist.tile([1, P], BF16, tag="ones_row_b",
                                      name="ones_row_b")
            nc.vector.memset(ones_row_b[:], 1.0)
            b2row = persist.tile([1, CDIM], BF16, tag="b2row", name="b2row")

        xq_sb = persist.tile([P, 8, HWDIM], FP8, tag="xq", name="xq_sb")
        xnq_sb = persist.tile([P, 8, HWDIM], FP8, tag="xnq", name="xnq_sb")
        x0q_sb = persist.tile([P, 8, HWDIM], FP8, tag="x0q", name="x0q_sb")
        w1q_sb = persist.tile([P, 8, FDIM], FP8, tag="w1q", name="w1q_sb")
        w2q_sb = persist.tile([P, 2, CDIM], FP8, tag="w2q", name="w2q_sb")
        b1t = [persist.tile([P, 1], F32, tag=f"b1t{i}", name=f"b1t{i}")
               for i in range(2)]
        h1q = persist.tile([P, 2, HWDIM], FP8, tag="h1q", name="h1q")
        m_q = [persist.tile([P, 2, CDIM], FP8, tag=f"m{i}", name=f"m{i}")
               for i in range(4)]
        # P^T as one [128, mt, i] tile: per-i-tile chunks write [:, :, it*128]
        # slabs with a single strided copy, so pt copies pipeline behind each
        # sigmoid inside the window instead of serializing after it
        pt_all = persist.tile([P, 8, HWDIM], FP8, tag="pt_all", name="pt_all")
        gt_q = [persist.tile([P, 2, HWDIM], FP8, tag=f"gt{i}", name=f"gt{i}")
                for i in range(4)]
        pmask = [persist.tile([P, HWDIM], BF16, tag=f"pm{i}", name=f"pm{i}")
                 for i in range(8)]
        rt_sb = [persist.tile([P, CDIM], BF16, tag=f"rt{i}", name=f"rt{i}")
                 for i in range(8)]
        e_sb = [persist.tile([P, JH], BF16, tag=f"e{i}", name=f"e{i}")
                for i in range(8)]
        inv_f = persist.tile([1, JH], F32, tag="inv_f", name="inv_f")
        invbcf = persist.tile([P, JH], F32, tag="invbcf", name="invbcf")
        cs_row = persist.tile([1, JH], BF16, tag="cs_row", name="cs_row")
        csbc = persist.tile([P, JH], BF16, tag="csbc", name="csbc")
        dd = persist.tile([P, P], BF16, tag="dd", name="dd")

        # ---- preload the sigmoid table set during the DMA window; it also
        # contains Copy+Relu, so the whole S window needs no further loads.
        # The A phase switches once to exp_and_others (Copy+Exp+Relu); Ln is
        # never used (reciprocal_approx_fast instead), so 2 loads total.
        warm_act = persist.tile([1, 2], F32, tag="warm_act", name="warm_act")
        nc.vector.memset(warm_act[:], 1.0)
        nc.scalar.activation(warm_act[0:1, :], warm_act[0:1, :], Sigmoid)

        # ---- input DMAs: xq/xnq first (S blocks on all of both) ----
        nc.sync.dma_start(xq_sb[:], xq[:, :, :])
        nc.sync.dma_start(xnq_sb[:], xnq[:, :, :])
        nc.sync.dma_start(x0q_sb[:], x0q[:, :, :])
        nc.sync.dma_start(w1q_sb[:], w1q[:, :, :])
        nc.sync.dma_start(w2q_sb[:], w2q[:, :, :])
        for i in range(2):
            nc.sync.dma_start(b1t[i][:], b1[i])
        if use_b2:
            nc.sync.dma_start(b2row[:], b2[:, :])
        for i in range(8):
            nc.sync.dma_start(rt_sb[i][:], rt[ts(i, P), :])

        with ExitStack() as s1:
            # ---- PE warmup during the DMA: transposes of the identity ----
            warm_scope = ExitStack()
            ps_w = warm_scope.enter_context(
                tc.tile_pool(name="ps_w", bufs=1, space="PSUM"))
            wt = ps_w.tile([P, P], BF16, tag="W")
            for _ in range(80):
                nc.tensor.transpose(wt[:], id_b[:], id_b[:])
            warm_scope.close()

            topk_pool = s1.enter_context(tc.tile_pool(name="topk", bufs=3))
            ps_sm = s1.enter_context(
                tc.tile_pool(name="ps_sm", bufs=2, space="PSUM"))
            ps_pt = s1.enter_context(
                tc.tile_pool(name="ps_pt", bufs=2, space="PSUM"))
            # ps_s is innermost so it can release its 4 banks (for ps_a/ps_cs)
            # while ps_sm/ps_pt live on through the G tail
            s_scope = ExitStack()
            ps_s = s_scope.enter_context(
                tc.tile_pool(name="ps_s", bufs=2, space="PSUM"))

            # ---- work chunks interleaved into the S/topk window ----
            def h_chunk(k):
                # h1[f,m] = relu(sum_c w1[c,f] x0[c,m] + b1[f]); f-block ft,
                # m-half ih
                ft, ih = k % 2, k // 2
                ps = ps_sm.tile([P, JH], F32, tag="SM", name="hps")
                for sp in range(4):
                    nc.tensor.matmul(
                        ps[:], w1q_sb[:, 2 * sp:2 * sp + 2, ts(ft, P)],
                        x0q_sb[:, 2 * sp:2 * sp + 2, ts(ih, JH)],
                        start=(sp == 0), stop=(sp == 3), perf_mode=DR,
                    )
                nc.scalar.activation(h1q[:, ft, ts(ih, JH)], ps[:], Relu,
                                     bias=b1t[ft][:])

            def m_chunk(k):
                # M[m,c] = relu(sum_f h1[f,m] w2[f,c] + b2[c]) / 16
                mb, ch = k // 2, k % 2
                ps = ps_sm.tile([P, JH], F32, tag="SM", name="mps")
                nc.tensor.matmul(ps[:], h1q[:, :, ts(mb, P)],
                                 w2q_sb[:, :, ts(ch, JH)],
                                 start=True, stop=not use_b2, perf_mode=DR)
                if use_b2:
                    nc.tensor.matmul(ps[:], ones_row_b[:],
                                     b2row[0:1, ts(ch, JH)],
                                     start=False, stop=True,
                                     skip_group_check=True)
                nc.scalar.activation(m_q[mb // 2][:, mb % 2, ts(ch, JH)],
                                     ps[:], Relu, scale=1.0 / 16.0)

            def pt_chunk(it, on_dve=False):
                # one i-tile of P^T: transpose pmask[it] against every
                # mt-block, then a single strided copy into the pt_all slab.
                # Depends only on pmask[it] -> pipelines inside the window.
                ps = ps_pt.tile([P, 8, P], BF16, tag="PT", name="ptps")
                for mt in range(8):
                    nc.tensor.transpose(ps[:, mt, :],
                                        pmask[it][:, ts(mt, P)], id_b[:])
                dst = pt_all[:, :, ts(it, P)]
                if on_dve:
                    nc.vector.tensor_copy(out=dst, in_=ps[:])
                else:
                    nc.scalar.activation(dst, ps[:], Copy)

            def g_chunk(ct, ih, on_dve=False):
                # G^T[c,i] = sum_m M[m,c] P^T[m,i]
                ps = ps_sm.tile([P, JH], F32, tag="SM", name="gps")
                for mp in range(4):
                    nc.tensor.matmul(ps[:], m_q[mp][:, :, ts(ct, P)],
                                     pt_all[:, 2 * mp:2 * mp + 2, ts(ih, JH)],
                                     start=(mp == 0), stop=(mp == 3),
                                     perf_mode=DR)
                dst = gt_q[ct // 2][:, ct % 2, ts(ih, JH)]
                if on_dve:
                    nc.vector.tensor_copy(out=dst, in_=ps[:])
                else:
                    nc.scalar.activation(dst, ps[:], Copy)

            # chunk order: H after x0q lands; PT(it) right after sigmoid(it)
            # (copies split DVE/ACT); G-ih0 needs PT 0..3 + the full M table
            H, M = h_chunk, m_chunk
            PT = lambda it: pt_chunk(it, on_dve=(it % 2 == 0))
            G0 = lambda ct: g_chunk(ct, 0, on_dve=(ct % 2 == 0))
            chunks = [
                lambda: H(0), lambda: H(1),                              # t1
                lambda: H(2), lambda: H(3), lambda: M(0), lambda: M(1),  # t2
                lambda: PT(0), lambda: M(2), lambda: M(3), lambda: M(4),  # t3
                lambda: PT(1), lambda: PT(2), lambda: M(5), lambda: M(6),
                lambda: M(7),                                            # t4
                lambda: PT(3), lambda: M(8), lambda: M(9), lambda: M(10),
                lambda: M(11),                                           # t5
                lambda: PT(4), lambda: PT(5), lambda: M(12), lambda: M(13),
                lambda: M(14), lambda: M(15), lambda: G0(0), lambda: G0(1),  # t6
                lambda: PT(6), lambda: G0(2), lambda: G0(3), lambda: G0(4),
                lambda: G0(5), lambda: G0(6), lambda: G0(7),             # t7
            ]
            sched = [0, 2, 4, 4, 5, 5, 8, 7]
            assert sum(sched) == len(chunks)

            # ---- S tiles + topk: the stationary operand comes from the
            # negated upload, so psum holds -S directly and the whole chain
            # (max8/match_replace/sigmoid) reads PSUM — no negate copies ----
            s_ps = [None] * 8

            def s_tile(t):
                ps = ps_s.tile([P, HWDIM], F32, tag="S")
                for jh in range(2):
                    for sp in range(4):
                        nc.tensor.matmul(
                            ps[:, ts(jh, JH)],
                            xnq_sb[:, 2 * sp:2 * sp + 2, ts(t, P)],
                            xq_sb[:, 2 * sp:2 * sp + 2, ts(jh, JH)],
                            start=(sp == 0), stop=(sp == 3), perf_mode=DR,
                        )
                s_ps[t] = ps

            def topk_tile(t):
                ps = s_ps[t]
                m8a = topk_pool.tile([P, 8], F32, tag="m8a", name="m8a")
                m8b = topk_pool.tile([P, 8], F32, tag="m8b", name="m8b")
                szap = topk_pool.tile([P, HWDIM], F32, tag="szap",
                                      name="szap", bufs=2)
                biast = topk_pool.tile([P, 1], F32, tag="biast", name="biast")
                nc.vector.max(out=m8a[:], in_=ps[:])
                nc.vector.match_replace(out=szap[:], in_to_replace=m8a[:],
                                        in_values=ps[:], imm_value=MINVAL)
                nc.vector.max(out=m8b[:], in_=szap[:])
                # bias = BIG*(DELTA - thr); mask = sigmoid(BIG*(-S) + bias).
                # The tiny [P,1] affine runs on GpSimd to keep it off the DVE
                # chain (DVE co-paces the window).
                nc.gpsimd.tensor_scalar(
                    out=biast[:], in0=m8b[:, 7:8], scalar1=-BIG,
                    scalar2=BIG * DELTA, op0=mybir.AluOpType.mult,
                    op1=mybir.AluOpType.add)
                nc.scalar.activation(pmask[t][:], ps[:], Sigmoid,
                                     bias=biast[:], scale=BIG)

            s_tile(0)
            for t in range(8):
                if t + 1 < 8:
                    s_tile(t + 1)
                topk_tile(t)
                for _ in range(sched[t]):
                    chunks.pop(0)()
            # hoist the sigmoid->exp table-set switch here: Copy stays valid
            # in both sets, so the load runs hidden behind the copy drain
            # instead of right before the first (critical) A-phase exp.
            # Reading pmask[7] pins this after the last sigmoid — the list
            # scheduler reorders dep-free instructions.
            nc.scalar.activation(warm_act[0:1, :], pmask[7][0:1, 0:2], Exp)
            for chunk in chunks:
                chunk()
            # S psum banks free; A can interleave with the ih=1 tail
            s_scope.close()

            ps_a = s1.enter_context(
                tc.tile_pool(name="ps_a", bufs=2, space="PSUM"))
            ps_cs = s1.enter_context(
                tc.tile_pool(name="ps_cs", bufs=1, space="PSUM"))
            ps_wf = s1.enter_context(
                tc.tile_pool(name="ps_wf", bufs=1, space="PSUM"))
            cs = ps_cs.tile([1, JH], F32, tag="CS")
            wf = ps_wf.tile([P, P], BF16, tag="WF", name="wf")

            def warm_fill(n, src=None):
                # PE transposes that keep the clock-gate ramp up while the
                # next real matmul waits on its inputs.  `src` adds a data
                # dep to pin them at this point in the schedule.
                src = id_b[:] if src is None else src
                for _ in range(n):
                    nc.tensor.transpose(wf[:], src, id_b[:])

            def a_chunk(mt):
                # A[m, own-j] = sum_c G^T[c,m] G^T[c,j];  exp -> e_sb
                ps = ps_a.tile([P, JH], F32, tag="A")
                for cp in range(4):
                    nc.tensor.matmul(ps[:], gt_q[cp][:, :, ts(mt, P)],
                                     gt_q[cp][:, :, 0:JH],
                                     start=(cp == 0), stop=(cp == 3),
                                     perf_mode=DR)
                nc.scalar.activation(e_sb[mt][:], ps[:], Exp)

            # only PT(7) remains after the window (needs the last sigmoid);
            # everything else of P^T already pipelined behind its own tile
            pt_chunk(7, on_dve=True)
            for mt in range(4):
                a_chunk(mt)
            for ct in range(8):
                g_chunk(ct, 1, on_dve=(ct % 2 == 0))
            for mt in range(4, 8):
                a_chunk(mt)
            # column sums batched at the end: no cs matmul ever blocks the
            # in-order PE queue mid-phase
            for mt in range(8):
                nc.tensor.matmul(cs[0:1, :], ones_col_b[:], e_sb[mt][:],
                                 start=(mt == 0), stop=(mt == 7))
            # keep the PE warm through the cs->csbc->diag-add latency
            warm_fill(24, src=e_sb[7][:, 0:P])

            # fold identity as E += diag(cs): out = (R^T (E + diag cs)) / cs
            # = X @ A_sm + X.  1/cs via fast DVE reciprocal (no Ln/Exp, no
            # table switches); inv applied post-psum per output tile.
            nc.vector.tensor_copy(out=cs_row[0:1, :], in_=cs[0:1, :])
            nc.gpsimd.partition_broadcast(csbc[:], cs_row[0:1, :], channels=P)
            nc.vector.reciprocal_approx_fast(inv_f[0:1, :], cs[0:1, :])
            nc.gpsimd.partition_broadcast(invbcf[:], inv_f[0:1, :], channels=P)
            for mt in range(4):
                nc.vector.tensor_tensor(out=dd[:], in0=id_b[:],
                                        in1=csbc[:, ts(mt, P)],
                                        op=mybir.AluOpType.mult)
                nc.vector.tensor_tensor(out=e_sb[mt][:, ts(mt, P)],
                                        in0=e_sb[mt][:, ts(mt, P)],
                                        in1=dd[:],
                                        op=mybir.AluOpType.add)

        # ---- OUT[ct] = sum_mt R^T E; finalize = psum * (1/cs) on DVE,
        # pipelined per ct against the matmul stream ----
        with ExitStack() as s5:
            ps_o = s5.enter_context(
                tc.tile_pool(name="ps_o", bufs=4, space="PSUM"))
            fin_pool = s5.enter_context(tc.tile_pool(name="fin", bufs=4))
            for ct in range(8):
                ps = ps_o.tile([P, JH], F32, tag="O")
                # mt 4..7 first: those e tiles have no diag add, so the row
                # can start before the cs->diag-add chain completes
                for mt in (4, 5, 6, 7, 0, 1, 2, 3):
                    nc.tensor.matmul(ps[:], rt_sb[mt][:, ts(ct, P)],
                                     e_sb[mt][:],
                                     start=(mt == 4), stop=(mt == 3))
                outt = fin_pool.tile([P, JH], F32, tag="outt", name="outt")
                nc.vector.tensor_tensor(out=outt[:], in0=ps[:], in1=invbcf[:],
                                        op=mybir.AluOpType.mult)
                nc.sync.dma_start(out[ts(ct, P), :], outt[:])

    return nc


_NC = {}


def _get_nc(use_b2=False):
    if use_b2 not in _NC:
        nc = bacc.Bacc("TRN2", target_bir_lowering=False, debug=False,
                       num_devices=NCORES)
        _build_program(nc, use_b2)
        nc.compile()
        _NC[use_b2] = nc
    return _NC[use_b2]


def _ksub(a):
    """[K, F] f32 -> [128, K//128, F] fp8 (ksub-major layout)."""
    K, F = a.shape
    return np.ascontiguousarray(
        a.reshape(K // P, P, F).transpose(1, 0, 2)).astype(
            ml_dtypes.float8_e4m3)


def _in_maps(cat, rgb_in, w1, b1, w2, b2):
    del cat  # unused by the reference computation
    x4 = np.ascontiguousarray(rgb_in.reshape(NB, CDIM, HWDIM)).astype(
        np.float32)
    w1 = np.ascontiguousarray(w1, dtype=np.float32)
    w2 = np.ascontiguousarray(w2, dtype=np.float32)
    b1r = np.ascontiguousarray(b1.reshape(2, P, 1), dtype=np.float32)
    b2r = np.ascontiguousarray(b2.reshape(1, CDIM)).astype(ml_dtypes.bfloat16)
    w1q = _ksub(w1)
    w2q = _ksub(w2)
    maps = []
    for core in range(NCORES):
        n, q = core // 2, core % 2
        # local node order: this core's 512 columns first
        roll = (lambda a: a) if q == 0 else (
            lambda a: np.ascontiguousarray(np.concatenate(
                [a[:, JH:], a[:, :JH]], axis=1)))
        xr = roll(x4[n])
        x0r = roll(x4[0])
        maps.append({
            "xq": _ksub(xr),
            "xnq": _ksub(-xr),
            "x0q": _ksub(x0r),
            "w1q": w1q,
            "w2q": w2q,
            "rt": np.ascontiguousarray(xr.T).astype(ml_dtypes.bfloat16),
            "b1": b1r,
            "b2": b2r,
        })
    return maps


def _assemble(results, rgb_shape):
    N, C, H, W = rgb_shape
    out = np.empty((N, C, H * W), np.float32)
    for core, res in enumerate(results):
        n, q = core // 2, core % 2
        out[n, :, q * JH:(q + 1) * JH] = res["out"]
    return out.reshape(N, C, H, W)


def run_on_hw(cat, rgb_in, w1, b1, w2, b2, trace=False, **kw):
    nc = _get_nc(use_b2=bool(np.any(np.asarray(b2))))
    maps = _in_maps(cat, rgb_in, w1, b1, w2, b2)
    res = run_bass_kernel_spmd(nc, maps, core_ids=list(range(NCORES)),
                               trace=trace, **kw)
    out = _assemble(res.results, rgb_in.shape)
    return out, res


def kernel(cat, rgb_in, w1, b1, w2, b2, gnn_iterations=1, k=16):
    assert int(gnn_iterations) == 1 and int(k) == 16
    cat = np.asarray(cat)
    rgb_in = np.asarray(rgb_in, dtype=np.float32)
    out, _ = run_on_hw(cat, rgb_in, np.asarray(w1), np.asarray(b1),
                       np.asarray(w2), np.asarray(b2))
    return out
